# revision 13
# baseline (speedup 1.0000x reference)
"""Multi-head self-attention on 8 TRN2 NeuronCores.

Strategy: hybrid shard — 2 batch groups x 4 head groups. Each core owns one
batch element (2048 tokens) and 4 of the 16 heads (a 256-column slice of
Wq/Wk/Wv and the matching 256-row slice of Wo). The host sums the 4 partial
outputs within each batch group (the tensor-parallel all-reduce) and adds bo.

Per-core kernel phases:
  1. QKV projections for the core's batch, outputs transposed [cols, tokens]
     in f16; V additionally transposed PE-side into [token, col] tiles with a
     ones-column appended (so the softmax normalizer falls out of the P@V
     matmul). Transposes are interleaved with projection matmuls so the PE
     never sees a long matmul-free stretch (keeps the HAM clock-gate warm).
  2. Attention: per (q-chunk, k-tile), scores for a head PAIR are written to
     one 2-bank PSUM tile ([128 k, 2x512 q]); exp runs 1024-wide on the
     scalar engine; P@V accumulates per head over k-tiles. Head pairs sit at
     partition bases 0/64 so their score matmuls overlap on the PE array.
  3. Normalizer: z rows (from the ones-column) are gathered 4-per-q-chunk,
     one batched reciprocal, broadcast across partitions via gpsimd, then
     fused multiply into the attention output.
  4. Output projection against the core's Wo row-slice; partial [D, S] f32
     written to DRAM.
"""

import numpy as np

B, S, D, H, DK = 2, 2048, 1024, 16, 64
NCORES = 8
HPC = 4                    # heads per core
COLS = HPC * DK            # feature columns per core = 256
NPH = COLS // 128          # partition tiles per core = 2
TCH = 512                  # token chunk (max fp32 moving free dim)
NT = S // TCH              # 4 token chunks
ND = D // 128              # 8 contraction chunks
NQ = S // TCH              # 4 query chunks
NKT = S // 128             # 16 key tiles
VW = 2 * (DK + 1)          # V' row width for a head pair = 130

_CACHE = {}


def _build_program():
    from contextlib import ExitStack

    import concourse.bacc as bacc
    import concourse.bass as bass
    import concourse.mybir as mybir
    import concourse.tile as tile
    from concourse.masks import make_identity

    f32 = mybir.dt.float32
    f32r = mybir.dt.float32r
    f16 = mybir.dt.float16
    Exp = mybir.ActivationFunctionType.Exp

    nc = bacc.Bacc("TRN2", target_bir_lowering=False, debug=False,
                   num_devices=NCORES)

    XT = nc.dram_tensor("xt", [D, S], f32, kind="ExternalInput").ap()
    Wq = nc.dram_tensor("wq", [D, COLS], f32, kind="ExternalInput").ap()
    Wk = nc.dram_tensor("wk", [D, COLS], f32, kind="ExternalInput").ap()
    Wv = nc.dram_tensor("wv", [D, COLS], f32, kind="ExternalInput").ap()
    Bq = nc.dram_tensor("bq", [COLS, 1], f32, kind="ExternalInput").ap()
    Bk = nc.dram_tensor("bk", [COLS, 1], f32, kind="ExternalInput").ap()
    Bv = nc.dram_tensor("bv", [COLS, 1], f32, kind="ExternalInput").ap()
    Wo = nc.dram_tensor("wo", [COLS, D], f32, kind="ExternalInput").ap()
    OT = nc.dram_tensor("ot", [D, S], f32, kind="ExternalOutput").ap()

    with tile.TileContext(nc) as tc, ExitStack() as ctx:
        consts = ctx.enter_context(tc.tile_pool(name="consts", bufs=1))
        xtp = ctx.enter_context(tc.tile_pool(name="xtp", bufs=16))
        big = ctx.enter_context(tc.tile_pool(name="big", bufs=1))
        expp = ctx.enter_context(tc.tile_pool(name="expp", bufs=4))
        arp = ctx.enter_context(tc.tile_pool(name="arp", bufs=8))
        zp = ctx.enter_context(tc.tile_pool(name="zp", bufs=2))
        zbp = ctx.enter_context(tc.tile_pool(name="zbp", bufs=4))
        osbp = ctx.enter_context(tc.tile_pool(name="osbp", bufs=4))
        drp = ctx.enter_context(tc.tile_pool(name="drp", bufs=2, space="DRAM"))
        # PSUM budget (8 banks): ps_s = 2 slots x 2 banks (scores ping-pong;
        # also QKV accumulation, warm-up and transposes), ps_patt = 2 slots
        # x 1 bank (attn accumulators, one head pair at a time), ps_o = 2
        # slots x 1 bank (output projection).
        ps_s = ctx.enter_context(tc.tile_pool(name="ps_s", bufs=2, space="PSUM"))
        ps_patt = ctx.enter_context(tc.tile_pool(name="ps_patt", bufs=2, space="PSUM"))
        ps_o = ctx.enter_context(tc.tile_pool(name="ps_o", bufs=2, space="PSUM"))

        # ---- weights for QKV first (phase 1 needs them before anything) ----
        w_sb = {}
        b_sb = {}
        for nm, src, bsrc in (("wq", Wq, Bq), ("wk", Wk, Bk), ("wv", Wv, Bv)):
            w_sb[nm] = []
            for d in range(ND):
                tiles = []
                for ph in range(NPH):
                    wt = consts.tile([128, 128], f32r, name=f"{nm}_{d}_{ph}")
                    nc.sync.dma_start(
                        wt, src[d * 128:(d + 1) * 128,
                                ph * 128:(ph + 1) * 128].bitcast(f32r))
                    tiles.append(wt)
                w_sb[nm].append(tiles)
            bts = []
            for ph in range(NPH):
                bt = consts.tile([128, 1], f32, name=f"b{nm[1]}_{ph}")
                nc.sync.dma_start(bt, bsrc[ph * 128:(ph + 1) * 128, :])
                bts.append(bt)
            b_sb[nm] = bts

        # Warm-up: dummy matmuls on the first weight tile keep the PE busy
        # while X streams in, so the HAM clock-gate reaches 8/8 before the
        # real work starts. Results are never read.
        for w in range(40):
            wrm = ps_s.tile([128, 128], f32, tag="s", name=f"warm_{w}")
            nc.tensor.matmul(wrm, lhsT=w_sb["wq"][0][0], rhs=w_sb["wq"][1][0],
                             start=True, stop=True)
        # Pre-load the exp activation table off the critical path.
        dume = consts.tile([128, 1], f16, name="dume")
        nc.scalar.activation(dume, b_sb["wq"][0], Exp)

        ident = consts.tile([128, 128], f16, name="ident")
        make_identity(nc, ident)

        # ---- phase 1: QKV projections + interleaved V transposes ----
        QTs = [big.tile([128, S], f16, name=f"QT_{ph}") for ph in range(NPH)]
        KTs = [big.tile([128, S], f16, name=f"KT_{ph}") for ph in range(NPH)]
        VTs = [big.tile([128, S], f16, name=f"VT_{ph}") for ph in range(NPH)]
        proj_out = {"wq": QTs, "wk": KTs, "wv": VTs}
        # V' tiles: per ph one [128, NKT*VW] f16 tile laid out per k-tile as
        # [Veven(64) | 1 | Vodd(64) | 1]; memset to 1.0 once so the ones
        # columns are in place before the strided copies fill the V parts.
        VPs = []
        for ph in range(NPH):
            vp = big.tile([128, NKT * VW], f16, name=f"VP_{ph}")
            nc.vector.memset(vp, 1.0)
            VPs.append(vp)

        for t in range(NT):
            xts = []
            for d in range(ND):
                xt_t = xtp.tile([128, TCH], f32r, name=f"xt_{t}_{d}", tag="xt")
                nc.gpsimd.dma_start(
                    xt_t,
                    XT[d * 128:(d + 1) * 128,
                       t * TCH:(t + 1) * TCH].bitcast(f32r))
                xts.append(xt_t)
            for nm in ("wq", "wk", "wv"):
                for ph in range(NPH):
                    pacc = ps_s.tile([128, TCH], f32, tag="s",
                                     name=f"pacc_{t}_{nm}_{ph}")
                    for d in range(ND):
                        nc.tensor.matmul(pacc, lhsT=w_sb[nm][d][ph],
                                         rhs=xts[d],
                                         start=(d == 0), stop=(d == ND - 1))
                    nc.vector.tensor_scalar_add(
                        proj_out[nm][ph][:, t * TCH:(t + 1) * TCH], pacc,
                        b_sb[nm][ph])
            # V transposes for this chunk: VT[ph][:, 128-token tile] ->
            # [token, col] then one strided copy into VP (cols 0:64 -> 0:64,
            # 64:128 -> 65:129), leaving the ones columns intact.
            for ph in range(NPH):
                for s4 in range(4):
                    kt = t * 4 + s4
                    ptr = ps_s.tile([128, 128], f16, tag="s",
                                    name=f"ptr_{ph}_{kt}")
                    nc.tensor.transpose(
                        ptr, VTs[ph][:, kt * 128:(kt + 1) * 128], ident)
                    dst = VPs[ph][:, kt * VW:(kt + 1) * VW]
                    dst2 = bass.AP(tensor=dst.tensor, offset=dst.offset,
                                   ap=dst.ap[:1] + [[DK + 1, 2], [1, DK]])
                    src = bass.AP(tensor=ptr.tensor, offset=ptr.offset,
                                  ap=ptr.ap[:1] + [[DK, 2], [1, DK]])
                    nc.vector.tensor_copy(dst2, src)

        # ---- Wo weights (needed later; loaded behind phase 1) ----
        wo_sb = [[None] * ND for _ in range(NPH)]
        for ph in range(NPH):
            for e in range(ND):
                wt = consts.tile([128, 128], f32r, name=f"wo_{ph}_{e}")
                nc.sync.dma_start(
                    wt, Wo[ph * 128:(ph + 1) * 128,
                           e * 128:(e + 1) * 128].bitcast(f32r))
                wo_sb[ph][e] = wt

        # ---- phase 2: attention ----
        attn_sb = [big.tile([128, S], f32r, name=f"attn_{ph}")
                   for ph in range(NPH)]
        for q in range(NQ):
            qs = slice(q * TCH, (q + 1) * TCH)
            zg = zp.tile([128, TCH], f32, tag="zg", name=f"zg_{q}")
            nc.vector.memset(zg, 1.0)
            araws = []
            # two passes: head pair (h0,h1) = ph 0, then (h2,h3) = ph 1
            for ph in range(NPH):
                patts = [ps_patt.tile([DK + 1, TCH], f32, tag="patt",
                                      name=f"patt_{q}_{ph}_{hp}")
                         for hp in range(2)]
                for kt in range(NKT):
                    sps = ps_s.tile([128, 2 * TCH], f32, tag="s",
                                    name=f"sc_{q}_{ph}_{kt}")
                    for hp in range(2):
                        rows = slice(hp * DK, (hp + 1) * DK)
                        nc.tensor.matmul(
                            sps[:, hp * TCH:(hp + 1) * TCH],
                            lhsT=KTs[ph][rows, kt * 128:(kt + 1) * 128],
                            rhs=QTs[ph][rows, qs],
                            start=True, stop=True)
                    esb = expp.tile([128, 2 * TCH], f16, tag="exp",
                                    name=f"exp_{q}_{ph}_{kt}")
                    nc.scalar.activation(esb, sps, Exp, scale=0.125)
                    for hp in range(2):
                        vcol = kt * VW + hp * (DK + 1)
                        nc.tensor.matmul(
                            patts[hp],
                            lhsT=VPs[ph][:, vcol:vcol + DK + 1],
                            rhs=esb[:, hp * TCH:(hp + 1) * TCH],
                            start=(kt == 0), stop=(kt == NKT - 1))
                # drain accumulators fast (frees the banks for the next
                # pass) and stash the normalizer rows at {0,32,64,96}.
                for hp in range(2):
                    i = 2 * ph + hp
                    ar = arp.tile([DK, TCH], f32, tag="araw",
                                  name=f"araw_{q}_{i}")
                    nc.vector.tensor_copy(ar, patts[hp][0:DK, :])
                    araws.append(ar)
                    nc.vector.tensor_copy(zg[32 * i:32 * i + 1, :],
                                          patts[hp][DK:DK + 1, :])
            zgr = zp.tile([128, TCH], f32, tag="zgr", name=f"zgr_{q}")
            nc.vector.reciprocal(zgr, zg)
            scratch = drp.tile([4, TCH], f32, tag="scr", name=f"scr_{q}")
            nc.gpsimd.dma_start(scratch, zgr[0:128:32, :])
            for i in range(4):
                ph, hp = i // 2, i % 2
                zbs = zbp.tile([DK, TCH], f32, tag="zbs", name=f"zbs_{q}_{i}")
                row = scratch[i:i + 1, :]
                nc.gpsimd.dma_start(
                    zbs,
                    bass.AP(tensor=row.tensor, offset=row.offset,
                            ap=[[0, DK]] + row.ap[1:]))
                nc.vector.tensor_mul(
                    attn_sb[ph][hp * DK:(hp + 1) * DK, qs], araws[i], zbs)
            # ---- output projection for this q chunk ----
            for e in range(ND):
                po = ps_o.tile([128, TCH], f32, tag="o", name=f"po_{q}_{e}")
                for ph in range(NPH):
                    nc.tensor.matmul(po, lhsT=wo_sb[ph][e],
                                     rhs=attn_sb[ph][:, qs],
                                     start=(ph == 0), stop=(ph == NPH - 1))
                ob = osbp.tile([128, TCH], f32, tag="osb", name=f"ob_{q}_{e}")
                nc.vector.tensor_copy(ob, po)
                nc.sync.dma_start(OT[e * 128:(e + 1) * 128, qs], ob)

    nc.compile()
    return nc


def _get_program():
    if "nc" not in _CACHE:
        _CACHE["nc"] = _build_program()
    return _CACHE["nc"]


def _install_ntff_hook():
    """Provide the antenv.axon_hooks shim this container's antenv lacks so
    run_bass_kernel_spmd(trace=True) can capture NTFF profiles."""
    import sys
    import types

    try:
        import antenv

        if hasattr(antenv, "axon_hooks"):
            return
        mod = types.ModuleType("antenv.axon_hooks")
        mod._hook = None
        mod.set_axon_ntff_profile_hook = lambda h: setattr(mod, "_hook", h)
        mod.get_axon_ntff_profile_hook = lambda: mod._hook
        sys.modules["antenv.axon_hooks"] = mod
        antenv.axon_hooks = mod
        from trn_agent_boot.trn_boot import _ntff_profile_via_ctypes

        mod.set_axon_ntff_profile_hook(
            _ntff_profile_via_ctypes("/opt/axon/libaxon_pjrt.so"))
    except Exception:
        pass


def kernel(X, Wq, bq, Wk, bk, Wv, bv, Wo, bo, _profile=False, _trace_cores=None):
    from concourse.bass_utils import run_bass_kernel_spmd

    if _profile:
        _install_ntff_hook()

    nc = _get_program()

    X = np.asarray(X, np.float32)
    XTb = [np.ascontiguousarray(X[b].T) for b in range(B)]
    Wq, Wk, Wv, Wo = (np.asarray(w, np.float32) for w in (Wq, Wk, Wv, Wo))
    bq, bk, bv, bo = (np.asarray(v, np.float32) for v in (bq, bk, bv, bo))

    in_maps = []
    for c in range(NCORES):
        b, g = divmod(c, NCORES // B)
        cs = slice(g * COLS, (g + 1) * COLS)
        in_maps.append({
            "xt": XTb[b],
            "wq": np.ascontiguousarray(Wq[:, cs]),
            "wk": np.ascontiguousarray(Wk[:, cs]),
            "wv": np.ascontiguousarray(Wv[:, cs]),
            "bq": np.ascontiguousarray(bq[cs].reshape(COLS, 1)),
            "bk": np.ascontiguousarray(bk[cs].reshape(COLS, 1)),
            "bv": np.ascontiguousarray(bv[cs].reshape(COLS, 1)),
            "wo": np.ascontiguousarray(Wo[cs, :]),
        })

    res = run_bass_kernel_spmd(
        nc, in_maps, core_ids=list(range(NCORES)),
        trace=_profile,
        trace_cores=(_trace_cores if _trace_cores is not None
                     else ([0] if _profile else None)),
    )

    G = NCORES // B
    outs = []
    for b in range(B):
        ot = res.results[b * G]["ot"].astype(np.float64)
        for g in range(1, G):
            ot += res.results[b * G + g]["ot"]
        outs.append((ot.T + bo).astype(np.float32))
    out = np.stack(outs).reshape(B, S, D)
    if _profile:
        kernel.last_exec_time_ns = res.exec_time_ns
        kernel.last_results = res
    return out


# revision 14
# speedup vs baseline: 1.0328x; 1.0328x over previous
"""Multi-head self-attention on 8 TRN2 NeuronCores.

Strategy: hybrid shard — 2 batch groups x 4 head groups. Each core owns one
batch element (2048 tokens) and 4 of the 16 heads (a 256-column slice of
Wq/Wk/Wv and the matching 256-row slice of Wo). The host sums the 4 partial
outputs within each batch group (the tensor-parallel all-reduce) and adds bo.

Per-core kernel phases:
  1. QKV projections for the core's batch, outputs transposed [cols, tokens]
     in f16; V additionally transposed PE-side into [token, col] tiles with a
     ones-column appended (so the softmax normalizer falls out of the P@V
     matmul). Transposes are interleaved with projection matmuls so the PE
     never sees a long matmul-free stretch (keeps the HAM clock-gate warm).
  2. Attention: per (q-chunk, k-tile), scores for a head PAIR are written to
     one 2-bank PSUM tile ([128 k, 2x512 q]); exp runs 1024-wide on the
     scalar engine; P@V accumulates per head over k-tiles. Head pairs sit at
     partition bases 0/64 so their score matmuls overlap on the PE array.
  3. Normalizer: z rows (from the ones-column) are gathered 4-per-q-chunk,
     one batched reciprocal, broadcast across partitions via gpsimd, then
     fused multiply into the attention output.
  4. Output projection against the core's Wo row-slice; partial [D, S] f32
     written to DRAM.
"""

import numpy as np

B, S, D, H, DK = 2, 2048, 1024, 16, 64
NCORES = 8
HPC = 4                    # heads per core
COLS = HPC * DK            # feature columns per core = 256
NPH = COLS // 128          # partition tiles per core = 2
TCH = 512                  # token chunk (max fp32 moving free dim)
NT = S // TCH              # 4 token chunks
ND = D // 128              # 8 contraction chunks
NQ = S // TCH              # 4 query chunks
NKT = S // 128             # 16 key tiles
VW = 2 * (DK + 1)          # V' row width for a head pair = 130

_CACHE = {}


def _build_program():
    from contextlib import ExitStack

    import concourse.bacc as bacc
    import concourse.bass as bass
    import concourse.mybir as mybir
    import concourse.tile as tile
    from concourse.masks import make_identity

    f32 = mybir.dt.float32
    f32r = mybir.dt.float32r
    f16 = mybir.dt.float16
    Exp = mybir.ActivationFunctionType.Exp

    nc = bacc.Bacc("TRN2", target_bir_lowering=False, debug=False,
                   num_devices=NCORES)

    XT = nc.dram_tensor("xt", [D, S], f16, kind="ExternalInput").ap()
    Wq = nc.dram_tensor("wq", [D, COLS], f16, kind="ExternalInput").ap()
    Wk = nc.dram_tensor("wk", [D, COLS], f16, kind="ExternalInput").ap()
    Wv = nc.dram_tensor("wv", [D, COLS], f16, kind="ExternalInput").ap()
    Bq = nc.dram_tensor("bq", [COLS, 1], f32, kind="ExternalInput").ap()
    Bk = nc.dram_tensor("bk", [COLS, 1], f32, kind="ExternalInput").ap()
    Bv = nc.dram_tensor("bv", [COLS, 1], f32, kind="ExternalInput").ap()
    Wo = nc.dram_tensor("wo", [COLS, D], f16, kind="ExternalInput").ap()
    OT = nc.dram_tensor("ot", [D, S], f32, kind="ExternalOutput").ap()

    with tile.TileContext(nc) as tc, ExitStack() as ctx:
        consts = ctx.enter_context(tc.tile_pool(name="consts", bufs=1))
        xtp = ctx.enter_context(tc.tile_pool(name="xtp", bufs=16))
        big = ctx.enter_context(tc.tile_pool(name="big", bufs=1))
        expp = ctx.enter_context(tc.tile_pool(name="expp", bufs=4))
        arp = ctx.enter_context(tc.tile_pool(name="arp", bufs=8))
        zp = ctx.enter_context(tc.tile_pool(name="zp", bufs=2))
        zbp = ctx.enter_context(tc.tile_pool(name="zbp", bufs=4))
        osbp = ctx.enter_context(tc.tile_pool(name="osbp", bufs=4))
        drp = ctx.enter_context(tc.tile_pool(name="drp", bufs=2, space="DRAM"))
        # PSUM budget (8 banks): ps_s = 2 slots x 2 banks (scores ping-pong;
        # also QKV accumulation, warm-up and transposes), ps_patt = 2 slots
        # x 1 bank (attn accumulators, one head pair at a time), ps_o = 2
        # slots x 1 bank (output projection).
        ps_s = ctx.enter_context(tc.tile_pool(name="ps_s", bufs=2, space="PSUM"))
        ps_patt = ctx.enter_context(tc.tile_pool(name="ps_patt", bufs=2, space="PSUM"))
        ps_o = ctx.enter_context(tc.tile_pool(name="ps_o", bufs=2, space="PSUM"))

        # ---- weights for QKV first (phase 1 needs them before anything) ----
        w_sb = {}
        b_sb = {}
        for nm, src, bsrc in (("wq", Wq, Bq), ("wk", Wk, Bk), ("wv", Wv, Bv)):
            w_sb[nm] = []
            for d in range(ND):
                tiles = []
                for ph in range(NPH):
                    wt = consts.tile([128, 128], f16, name=f"{nm}_{d}_{ph}")
                    nc.sync.dma_start(
                        wt, src[d * 128:(d + 1) * 128,
                                ph * 128:(ph + 1) * 128])
                    tiles.append(wt)
                w_sb[nm].append(tiles)
            bts = []
            for ph in range(NPH):
                bt = consts.tile([128, 1], f32, name=f"b{nm[1]}_{ph}")
                nc.sync.dma_start(bt, bsrc[ph * 128:(ph + 1) * 128, :])
                bts.append(bt)
            b_sb[nm] = bts

        # Warm-up: dummy matmuls on the first weight tile keep the PE busy
        # while X streams in, so the HAM clock-gate reaches 8/8 before the
        # real work starts. Results are never read.
        for w in range(40):
            wrm = ps_s.tile([128, 128], f32, tag="s", name=f"warm_{w}")
            nc.tensor.matmul(wrm, lhsT=w_sb["wq"][0][0], rhs=w_sb["wq"][1][0],
                             start=True, stop=True)
        # Pre-load the exp activation table off the critical path.
        dume = consts.tile([128, 1], f16, name="dume")
        nc.scalar.activation(dume, b_sb["wq"][0], Exp)

        ident = consts.tile([128, 128], f16, name="ident")
        make_identity(nc, ident)

        # ---- phase 1: QKV projections + interleaved V transposes ----
        QTs = [big.tile([128, S], f16, name=f"QT_{ph}") for ph in range(NPH)]
        KTs = [big.tile([128, S], f16, name=f"KT_{ph}") for ph in range(NPH)]
        VTs = [big.tile([128, S], f16, name=f"VT_{ph}") for ph in range(NPH)]
        proj_out = {"wq": QTs, "wk": KTs, "wv": VTs}
        # V' tiles: per ph one [128, NKT*VW] f16 tile laid out per k-tile as
        # [Veven(64) | 1 | Vodd(64) | 1]; memset to 1.0 once so the ones
        # columns are in place before the strided copies fill the V parts.
        VPs = []
        for ph in range(NPH):
            vp = big.tile([128, NKT * VW], f16, name=f"VP_{ph}")
            nc.vector.memset(vp, 1.0)
            VPs.append(vp)

        for t in range(NT):
            xts = []
            for d in range(ND):
                xt_t = xtp.tile([128, TCH], f16, name=f"xt_{t}_{d}", tag="xt")
                nc.gpsimd.dma_start(
                    xt_t,
                    XT[d * 128:(d + 1) * 128,
                       t * TCH:(t + 1) * TCH])
                xts.append(xt_t)
            for nm in ("wq", "wk", "wv"):
                for ph in range(NPH):
                    pacc = ps_s.tile([128, TCH], f32, tag="s",
                                     name=f"pacc_{t}_{nm}_{ph}")
                    for d in range(ND):
                        nc.tensor.matmul(pacc, lhsT=w_sb[nm][d][ph],
                                         rhs=xts[d],
                                         start=(d == 0), stop=(d == ND - 1))
                    nc.vector.tensor_scalar_add(
                        proj_out[nm][ph][:, t * TCH:(t + 1) * TCH], pacc,
                        b_sb[nm][ph])
            # V transposes for this chunk: VT[ph][:, 128-token tile] ->
            # [token, col] then one strided copy into VP (cols 0:64 -> 0:64,
            # 64:128 -> 65:129), leaving the ones columns intact.
            for ph in range(NPH):
                for s4 in range(4):
                    kt = t * 4 + s4
                    ptr = ps_s.tile([128, 128], f16, tag="s",
                                    name=f"ptr_{ph}_{kt}")
                    nc.tensor.transpose(
                        ptr, VTs[ph][:, kt * 128:(kt + 1) * 128], ident)
                    dst = VPs[ph][:, kt * VW:(kt + 1) * VW]
                    dst2 = bass.AP(tensor=dst.tensor, offset=dst.offset,
                                   ap=dst.ap[:1] + [[DK + 1, 2], [1, DK]])
                    src = bass.AP(tensor=ptr.tensor, offset=ptr.offset,
                                  ap=ptr.ap[:1] + [[DK, 2], [1, DK]])
                    nc.vector.tensor_copy(dst2, src)

        # ---- Wo weights (needed later; loaded behind phase 1) ----
        wo_sb = [[None] * ND for _ in range(NPH)]
        for ph in range(NPH):
            for e in range(ND):
                wt = consts.tile([128, 128], f16, name=f"wo_{ph}_{e}")
                nc.sync.dma_start(
                    wt, Wo[ph * 128:(ph + 1) * 128,
                           e * 128:(e + 1) * 128])
                wo_sb[ph][e] = wt

        # ---- phase 2: attention ----
        attn_sb = [big.tile([128, S], f16, name=f"attn_{ph}")
                   for ph in range(NPH)]
        for q in range(NQ):
            qs = slice(q * TCH, (q + 1) * TCH)
            zg = zp.tile([128, TCH], f32, tag="zg", name=f"zg_{q}")
            nc.vector.memset(zg, 1.0)
            araws = []
            # two passes: head pair (h0,h1) = ph 0, then (h2,h3) = ph 1
            for ph in range(NPH):
                patts = [ps_patt.tile([DK + 1, TCH], f32, tag="patt",
                                      name=f"patt_{q}_{ph}_{hp}")
                         for hp in range(2)]
                for kt in range(NKT):
                    sps = ps_s.tile([128, 2 * TCH], f32, tag="s",
                                    name=f"sc_{q}_{ph}_{kt}")
                    for hp in range(2):
                        rows = slice(hp * DK, (hp + 1) * DK)
                        nc.tensor.matmul(
                            sps[:, hp * TCH:(hp + 1) * TCH],
                            lhsT=KTs[ph][rows, kt * 128:(kt + 1) * 128],
                            rhs=QTs[ph][rows, qs],
                            start=True, stop=True)
                    esb = expp.tile([128, 2 * TCH], f16, tag="exp",
                                    name=f"exp_{q}_{ph}_{kt}")
                    nc.scalar.activation(esb, sps, Exp, scale=0.125)
                    for hp in range(2):
                        vcol = kt * VW + hp * (DK + 1)
                        nc.tensor.matmul(
                            patts[hp],
                            lhsT=VPs[ph][:, vcol:vcol + DK + 1],
                            rhs=esb[:, hp * TCH:(hp + 1) * TCH],
                            start=(kt == 0), stop=(kt == NKT - 1))
                # drain accumulators fast (frees the banks for the next
                # pass) and stash the normalizer rows at {0,32,64,96}.
                for hp in range(2):
                    i = 2 * ph + hp
                    ar = arp.tile([DK, TCH], f32, tag="araw",
                                  name=f"araw_{q}_{i}")
                    nc.vector.tensor_copy(ar, patts[hp][0:DK, :])
                    araws.append(ar)
                    nc.vector.tensor_copy(zg[32 * i:32 * i + 1, :],
                                          patts[hp][DK:DK + 1, :])
            zgr = zp.tile([128, TCH], f32, tag="zgr", name=f"zgr_{q}")
            nc.vector.reciprocal(zgr, zg)
            scratch = drp.tile([4, TCH], f32, tag="scr", name=f"scr_{q}")
            nc.gpsimd.dma_start(scratch, zgr[0:128:32, :])
            for i in range(4):
                ph, hp = i // 2, i % 2
                zbs = zbp.tile([DK, TCH], f32, tag="zbs", name=f"zbs_{q}_{i}")
                row = scratch[i:i + 1, :]
                nc.gpsimd.dma_start(
                    zbs,
                    bass.AP(tensor=row.tensor, offset=row.offset,
                            ap=[[0, DK]] + row.ap[1:]))
                nc.vector.tensor_mul(
                    attn_sb[ph][hp * DK:(hp + 1) * DK, qs], araws[i], zbs)
            # ---- output projection for this q chunk ----
            for e in range(ND):
                po = ps_o.tile([128, TCH], f32, tag="o", name=f"po_{q}_{e}")
                for ph in range(NPH):
                    nc.tensor.matmul(po, lhsT=wo_sb[ph][e],
                                     rhs=attn_sb[ph][:, qs],
                                     start=(ph == 0), stop=(ph == NPH - 1))
                ob = osbp.tile([128, TCH], f32, tag="osb", name=f"ob_{q}_{e}")
                nc.vector.tensor_copy(ob, po)
                nc.sync.dma_start(OT[e * 128:(e + 1) * 128, qs], ob)

    nc.compile()
    return nc


def _get_program():
    if "nc" not in _CACHE:
        _CACHE["nc"] = _build_program()
    return _CACHE["nc"]


def _install_ntff_hook():
    """Provide the antenv.axon_hooks shim this container's antenv lacks so
    run_bass_kernel_spmd(trace=True) can capture NTFF profiles."""
    import sys
    import types

    try:
        import antenv

        if hasattr(antenv, "axon_hooks"):
            return
        mod = types.ModuleType("antenv.axon_hooks")
        mod._hook = None
        mod.set_axon_ntff_profile_hook = lambda h: setattr(mod, "_hook", h)
        mod.get_axon_ntff_profile_hook = lambda: mod._hook
        sys.modules["antenv.axon_hooks"] = mod
        antenv.axon_hooks = mod
        from trn_agent_boot.trn_boot import _ntff_profile_via_ctypes

        mod.set_axon_ntff_profile_hook(
            _ntff_profile_via_ctypes("/opt/axon/libaxon_pjrt.so"))
    except Exception:
        pass


def kernel(X, Wq, bq, Wk, bk, Wv, bv, Wo, bo, _profile=False, _trace_cores=None):
    from concourse.bass_utils import run_bass_kernel_spmd

    if _profile:
        _install_ntff_hook()

    nc = _get_program()

    X = np.asarray(X, np.float32)
    XTb = [np.ascontiguousarray(X[b].T.astype(np.float16)) for b in range(B)]
    Wq, Wk, Wv, Wo = (np.asarray(w, np.float16) for w in (Wq, Wk, Wv, Wo))
    bq, bk, bv, bo = (np.asarray(v, np.float32) for v in (bq, bk, bv, bo))

    in_maps = []
    for c in range(NCORES):
        b, g = divmod(c, NCORES // B)
        cs = slice(g * COLS, (g + 1) * COLS)
        in_maps.append({
            "xt": XTb[b],
            "wq": np.ascontiguousarray(Wq[:, cs]),
            "wk": np.ascontiguousarray(Wk[:, cs]),
            "wv": np.ascontiguousarray(Wv[:, cs]),
            "bq": np.ascontiguousarray(bq[cs].reshape(COLS, 1)),
            "bk": np.ascontiguousarray(bk[cs].reshape(COLS, 1)),
            "bv": np.ascontiguousarray(bv[cs].reshape(COLS, 1)),
            "wo": np.ascontiguousarray(Wo[cs, :]),
        })

    res = run_bass_kernel_spmd(
        nc, in_maps, core_ids=list(range(NCORES)),
        trace=_profile,
        trace_cores=(_trace_cores if _trace_cores is not None
                     else ([0] if _profile else None)),
    )

    G = NCORES // B
    outs = []
    for b in range(B):
        ot = res.results[b * G]["ot"].astype(np.float64)
        for g in range(1, G):
            ot += res.results[b * G + g]["ot"]
        outs.append((ot.T + bo).astype(np.float32))
    out = np.stack(outs).reshape(B, S, D)
    if _profile:
        kernel.last_exec_time_ns = res.exec_time_ns
        kernel.last_results = res
    return out


# revision 19
# speedup vs baseline: 1.2865x; 1.2456x over previous
"""Multi-head self-attention on 8 TRN2 NeuronCores.

Strategy: hybrid shard — 2 batch groups x 4 head groups. Each core owns one
batch element (2048 tokens) and 4 of the 16 heads (a 256-column slice of
Wq/Wk/Wv and the matching 256-row slice of Wo). The host sums the 4 partial
outputs within each batch group (the tensor-parallel all-reduce) and adds bo.

Per-core kernel phases:
  1. QKV projections for the core's batch, outputs transposed [cols, tokens]
     in f16; V additionally transposed PE-side into [token, col] tiles with a
     ones-column appended (so the softmax normalizer falls out of the P@V
     matmul). Transposes are interleaved with projection matmuls so the PE
     never sees a long matmul-free stretch (keeps the HAM clock-gate warm).
  2. Attention: per (q-chunk, k-tile), scores for a head PAIR are written to
     one 2-bank PSUM tile ([128 k, 2x512 q]); exp runs 1024-wide on the
     scalar engine; P@V accumulates per head over k-tiles. Head pairs sit at
     partition bases 0/64 so their score matmuls overlap on the PE array.
  3. Normalizer: z rows (from the ones-column) are gathered 4-per-q-chunk,
     one batched reciprocal, broadcast across partitions via gpsimd, then
     fused multiply into the attention output.
  4. Output projection against the core's Wo row-slice; partial [D, S] f32
     written to DRAM.
"""

import numpy as np

B, S, D, H, DK = 2, 2048, 1024, 16, 64
NCORES = 8
HPC = 4                    # heads per core
COLS = HPC * DK            # feature columns per core = 256
NPH = COLS // 128          # partition tiles per core = 2
TCH = 512                  # token chunk (max fp32 moving free dim)
NT = S // TCH              # 4 token chunks
ND = D // 128              # 8 contraction chunks
NQ = S // TCH              # 4 query chunks
NKT = S // 128             # 16 key tiles
VW = 2 * (DK + 1)          # V' row width for a head pair = 130

_CACHE = {}


def _build_program():
    from contextlib import ExitStack

    import concourse.bacc as bacc
    import concourse.bass as bass
    import concourse.mybir as mybir
    import concourse.tile as tile
    from concourse.masks import make_identity

    f32 = mybir.dt.float32
    f32r = mybir.dt.float32r
    f16 = mybir.dt.float16
    Exp = mybir.ActivationFunctionType.Exp

    nc = bacc.Bacc("TRN2", target_bir_lowering=False, debug=False,
                   num_devices=NCORES)

    XT = nc.dram_tensor("xt", [D, S], f16, kind="ExternalInput").ap()
    Wq = nc.dram_tensor("wq", [D, COLS], f16, kind="ExternalInput").ap()
    Wk = nc.dram_tensor("wk", [D, COLS], f16, kind="ExternalInput").ap()
    Wv = nc.dram_tensor("wv", [D, COLS], f16, kind="ExternalInput").ap()
    Bq = nc.dram_tensor("bq", [COLS, 1], f32, kind="ExternalInput").ap()
    Bk = nc.dram_tensor("bk", [COLS, 1], f32, kind="ExternalInput").ap()
    Bv = nc.dram_tensor("bv", [COLS, 1], f32, kind="ExternalInput").ap()
    Wo = nc.dram_tensor("wo", [COLS, D], f16, kind="ExternalInput").ap()
    OT = nc.dram_tensor("ot", [D, S], f32, kind="ExternalOutput").ap()

    with tile.TileContext(nc) as tc, ExitStack() as ctx:
        consts = ctx.enter_context(tc.tile_pool(name="consts", bufs=1))
        xtp = ctx.enter_context(tc.tile_pool(name="xtp", bufs=16))
        big = ctx.enter_context(tc.tile_pool(name="big", bufs=1))
        expp = ctx.enter_context(tc.tile_pool(name="expp", bufs=6))
        arp = ctx.enter_context(tc.tile_pool(name="arp", bufs=8))
        zp = ctx.enter_context(tc.tile_pool(name="zp", bufs=2))
        zbp = ctx.enter_context(tc.tile_pool(name="zbp", bufs=4))
        osbp = ctx.enter_context(tc.tile_pool(name="osbp", bufs=4))
        drp = ctx.enter_context(tc.tile_pool(name="drp", bufs=2, space="DRAM"))
        # PSUM budget (8 banks): ps_s = 2 slots x 2 banks (scores ping-pong;
        # also QKV accumulation, warm-up and transposes), ps_patt = 2 slots
        # x 1 bank (attn accumulators, one head pair at a time), ps_o = 2
        # slots x 1 bank (output projection).
        ps_s = ctx.enter_context(tc.tile_pool(name="ps_s", bufs=2, space="PSUM"))
        ps_patt = ctx.enter_context(tc.tile_pool(name="ps_patt", bufs=2, space="PSUM"))
        ps_o = ctx.enter_context(tc.tile_pool(name="ps_o", bufs=2, space="PSUM"))

        # ---- weights for QKV first (phase 1 needs them before anything).
        # One batched DMA per (proj, ph): [128, 8*128] with the 8 contraction
        # chunks side by side (2KB partition lines instead of 64 x 256B).
        w_sb = {}
        b_sb = {}
        for nm, src, bsrc in (("wq", Wq, Bq), ("wk", Wk, Bk), ("wv", Wv, Bv)):
            w_sb[nm] = []
            for ph in range(NPH):
                wt = consts.tile([128, ND * 128], f16, name=f"{nm}_{ph}")
                nc.sync.dma_start(
                    wt,
                    bass.AP(tensor=src.tensor, offset=ph * 128,
                            ap=[[COLS, 128], [128 * COLS, ND], [1, 128]]))
                w_sb[nm].append(wt)
            bts = []
            for ph in range(NPH):
                bt = consts.tile([128, 1], f32, name=f"b{nm[1]}_{ph}")
                nc.sync.dma_start(bt, bsrc[ph * 128:(ph + 1) * 128, :])
                bts.append(bt)
            b_sb[nm] = bts

        # Warm-up: dummy matmuls on the first weight tile keep the PE busy
        # while X streams in, so the HAM clock-gate reaches 8/8 before the
        # real work starts. Results are never read.
        for w in range(40):
            wrm = ps_s.tile([128, 128], f32, tag="s", name=f"warm_{w}")
            nc.tensor.matmul(wrm, lhsT=w_sb["wq"][0][:, 0:128],
                             rhs=w_sb["wq"][1][:, 0:128],
                             start=True, stop=True)
        # Pre-load the exp activation table off the critical path.
        dume = consts.tile([128, 1], f16, name="dume")
        nc.scalar.activation(dume, b_sb["wq"][0], Exp)

        ident = consts.tile([128, 128], f16, name="ident")
        make_identity(nc, ident)

        # ---- phase 1: QKV projections + interleaved V transposes ----
        QTs = [big.tile([128, S], f16, name=f"QT_{ph}") for ph in range(NPH)]
        KTs = [big.tile([128, S], f16, name=f"KT_{ph}") for ph in range(NPH)]
        VTs = [big.tile([128, S], f16, name=f"VT_{ph}") for ph in range(NPH)]
        proj_out = {"wq": QTs, "wk": KTs, "wv": VTs}
        # V' tiles: per ph one [128, NKT*VW] f16 tile laid out per k-tile as
        # [Veven(64) | 1 | Vodd(64) | 1]; memset to 1.0 once so the ones
        # columns are in place before the strided copies fill the V parts.
        VPs = []
        for ph in range(NPH):
            vp = big.tile([128, NKT * VW], f16, name=f"VP_{ph}")
            nc.vector.memset(vp, 1.0)
            VPs.append(vp)

        for t in range(NT):
            xts = []
            for d in range(ND):
                xt_t = xtp.tile([128, TCH], f16, name=f"xt_{t}_{d}", tag="xt")
                nc.gpsimd.dma_start(
                    xt_t,
                    XT[d * 128:(d + 1) * 128,
                       t * TCH:(t + 1) * TCH])
                xts.append(xt_t)
            for nm in ("wq", "wk", "wv"):
                for ph in range(NPH):
                    pacc = ps_s.tile([128, TCH], f32, tag="s",
                                     name=f"pacc_{t}_{nm}_{ph}")
                    for d in range(ND):
                        nc.tensor.matmul(pacc,
                                         lhsT=w_sb[nm][ph][:, d * 128:(d + 1) * 128],
                                         rhs=xts[d],
                                         start=(d == 0), stop=(d == ND - 1))
                    nc.vector.tensor_scalar_add(
                        proj_out[nm][ph][:, t * TCH:(t + 1) * TCH], pacc,
                        b_sb[nm][ph])
            # V transposes for this chunk: VT[ph][:, 128-token tile] ->
            # [token, col] then one strided copy into VP (cols 0:64 -> 0:64,
            # 64:128 -> 65:129), leaving the ones columns intact.
            for ph in range(NPH):
                for s4 in range(4):
                    kt = t * 4 + s4
                    ptr = ps_s.tile([128, 128], f16, tag="s",
                                    name=f"ptr_{ph}_{kt}")
                    nc.tensor.transpose(
                        ptr, VTs[ph][:, kt * 128:(kt + 1) * 128], ident)
                    dst = VPs[ph][:, kt * VW:(kt + 1) * VW]
                    dst2 = bass.AP(tensor=dst.tensor, offset=dst.offset,
                                   ap=dst.ap[:1] + [[DK + 1, 2], [1, DK]])
                    src = bass.AP(tensor=ptr.tensor, offset=ptr.offset,
                                  ap=ptr.ap[:1] + [[DK, 2], [1, DK]])
                    nc.vector.tensor_copy(dst2, src)

        # ---- Wo weights (needed later; loaded behind phase 1) ----
        wo_sb = []
        for ph in range(NPH):
            wt = consts.tile([128, ND * 128], f16, name=f"wo_{ph}")
            nc.sync.dma_start(
                wt,
                bass.AP(tensor=Wo.tensor, offset=ph * 128 * D,
                        ap=[[D, 128], [128, ND], [1, 128]]))
            wo_sb.append(wt)

        # ---- phase 2: attention ----
        attn_sb = [big.tile([128, S], f16, name=f"attn_{ph}")
                   for ph in range(NPH)]
        for q in range(NQ):
            qs = slice(q * TCH, (q + 1) * TCH)
            zg = zp.tile([128, TCH], f32, tag="zg", name=f"zg_{q}")
            nc.vector.memset(zg, 1.0)
            araws = []
            # two passes: head pair (h0,h1) = ph 0, then (h2,h3) = ph 1
            for ph in range(NPH):
                patts = [ps_patt.tile([DK + 1, TCH], f32, tag="patt",
                                      name=f"patt_{q}_{ph}_{hp}")
                         for hp in range(2)]
                for kt in range(NKT):
                    sps = ps_s.tile([128, 2 * TCH], f32, tag="s",
                                    name=f"sc_{q}_{ph}_{kt}")
                    for hp in range(2):
                        rows = slice(hp * DK, (hp + 1) * DK)
                        nc.tensor.matmul(
                            sps[:, hp * TCH:(hp + 1) * TCH],
                            lhsT=KTs[ph][rows, kt * 128:(kt + 1) * 128],
                            rhs=QTs[ph][rows, qs],
                            start=True, stop=True)
                    esb = expp.tile([128, 2 * TCH], f16, tag="exp",
                                    name=f"exp_{q}_{ph}_{kt}")
                    nc.scalar.activation(esb, sps, Exp, scale=0.125)
                    for hp in range(2):
                        vcol = kt * VW + hp * (DK + 1)
                        nc.tensor.matmul(
                            patts[hp],
                            lhsT=VPs[ph][:, vcol:vcol + DK + 1],
                            rhs=esb[:, hp * TCH:(hp + 1) * TCH],
                            start=(kt == 0), stop=(kt == NKT - 1))
                # drain accumulators fast (frees the banks for the next
                # pass) and stash the normalizer rows at {0,32,64,96}.
                for hp in range(2):
                    i = 2 * ph + hp
                    ar = arp.tile([DK, TCH], f32, tag="araw",
                                  name=f"araw_{q}_{i}")
                    nc.vector.tensor_copy(ar, patts[hp][0:DK, :])
                    araws.append(ar)
                    nc.vector.tensor_copy(zg[32 * i:32 * i + 1, :],
                                          patts[hp][DK:DK + 1, :])
            zgr = zp.tile([128, TCH], f32, tag="zgr", name=f"zgr_{q}")
            nc.vector.reciprocal(zgr, zg)
            scratch = drp.tile([4, TCH], f32, tag="scr", name=f"scr_{q}")
            nc.gpsimd.dma_start(scratch, zgr[0:128:32, :])
            for i in range(4):
                ph, hp = i // 2, i % 2
                zbs = zbp.tile([DK, TCH], f32, tag="zbs", name=f"zbs_{q}_{i}")
                row = scratch[i:i + 1, :]
                nc.gpsimd.dma_start(
                    zbs,
                    bass.AP(tensor=row.tensor, offset=row.offset,
                            ap=[[0, DK]] + row.ap[1:]))
                nc.vector.tensor_mul(
                    attn_sb[ph][hp * DK:(hp + 1) * DK, qs], araws[i], zbs)
            # ---- output projection for this q chunk ----
            for e in range(ND):
                po = ps_o.tile([128, TCH], f32, tag="o", name=f"po_{q}_{e}")
                for ph in range(NPH):
                    nc.tensor.matmul(po,
                                     lhsT=wo_sb[ph][:, e * 128:(e + 1) * 128],
                                     rhs=attn_sb[ph][:, qs],
                                     start=(ph == 0), stop=(ph == NPH - 1))
                ob = osbp.tile([128, TCH], f32, tag="osb", name=f"ob_{q}_{e}")
                nc.vector.tensor_copy(ob, po)
                nc.sync.dma_start(OT[e * 128:(e + 1) * 128, qs], ob)

    nc.compile()
    return nc


def _get_program():
    if "nc" not in _CACHE:
        _CACHE["nc"] = _build_program()
    return _CACHE["nc"]


def _install_ntff_hook():
    """Provide the antenv.axon_hooks shim this container's antenv lacks so
    run_bass_kernel_spmd(trace=True) can capture NTFF profiles."""
    import sys
    import types

    try:
        import antenv

        if hasattr(antenv, "axon_hooks"):
            return
        mod = types.ModuleType("antenv.axon_hooks")
        mod._hook = None
        mod.set_axon_ntff_profile_hook = lambda h: setattr(mod, "_hook", h)
        mod.get_axon_ntff_profile_hook = lambda: mod._hook
        sys.modules["antenv.axon_hooks"] = mod
        antenv.axon_hooks = mod
        from trn_agent_boot.trn_boot import _ntff_profile_via_ctypes

        mod.set_axon_ntff_profile_hook(
            _ntff_profile_via_ctypes("/opt/axon/libaxon_pjrt.so"))
    except Exception:
        pass


def kernel(X, Wq, bq, Wk, bk, Wv, bv, Wo, bo, _profile=False, _trace_cores=None):
    from concourse.bass_utils import run_bass_kernel_spmd

    if _profile:
        _install_ntff_hook()

    nc = _get_program()

    X = np.asarray(X, np.float32)
    XTb = [np.ascontiguousarray(X[b].T.astype(np.float16)) for b in range(B)]
    Wq, Wk, Wv, Wo = (np.asarray(w, np.float16) for w in (Wq, Wk, Wv, Wo))
    bq, bk, bv, bo = (np.asarray(v, np.float32) for v in (bq, bk, bv, bo))

    in_maps = []
    for c in range(NCORES):
        b, g = divmod(c, NCORES // B)
        cs = slice(g * COLS, (g + 1) * COLS)
        in_maps.append({
            "xt": XTb[b],
            "wq": np.ascontiguousarray(Wq[:, cs]),
            "wk": np.ascontiguousarray(Wk[:, cs]),
            "wv": np.ascontiguousarray(Wv[:, cs]),
            "bq": np.ascontiguousarray(bq[cs].reshape(COLS, 1)),
            "bk": np.ascontiguousarray(bk[cs].reshape(COLS, 1)),
            "bv": np.ascontiguousarray(bv[cs].reshape(COLS, 1)),
            "wo": np.ascontiguousarray(Wo[cs, :]),
        })

    res = run_bass_kernel_spmd(
        nc, in_maps, core_ids=list(range(NCORES)),
        trace=_profile,
        trace_cores=(_trace_cores if _trace_cores is not None
                     else ([0] if _profile else None)),
    )

    G = NCORES // B
    outs = []
    for b in range(B):
        ot = res.results[b * G]["ot"].astype(np.float64)
        for g in range(1, G):
            ot += res.results[b * G + g]["ot"]
        outs.append((ot.T + bo).astype(np.float32))
    out = np.stack(outs).reshape(B, S, D)
    if _profile:
        kernel.last_exec_time_ns = res.exec_time_ns
        kernel.last_results = res
    return out
